# revision 1
# baseline (speedup 1.0000x reference)
"""Trainium2 Bass kernel for nn_DMPNN_Change_678604832935 (8-core SPMD DMPNN+Set2Set).

Sharding: each core owns 64 consecutive graphs (batch is sorted) plus all edges
whose dst node falls in those graphs — so segment_sum is core-local and no
collectives are needed.  The node-side product z = relu(x@W0+b0) @ Wm1[:D] is
computed replicated on every core into its own HBM; per-edge rows are fetched
with dma_gather.  Since segment_sum is linear and sits between the Wm2 matmul
and the root update, m@Wm2 is folded to the node side (16x fewer FLOPs), with
deg(n)*bm2 as a rank-1 correction.  Set2Set runs in a [graph, 128-slot] grid
layout; softmax uses unnormalized exp (|e|<~8, validated) and sigmoid is
synthesized from tanh so the whole kernel uses one ACT table set.
"""

import os
import sys

for _p in ("/opt/trn_rl_repo", "/root/.axon_site/_ro/trn_rl_repo"):
    if os.path.isdir(_p) and _p not in sys.path:
        sys.path.append(_p)

import numpy as np

import concourse.bass as bass
import concourse.bacc as bacc
import concourse.mybir as mybir
import concourse.tile as tile
from concourse.bass_utils import run_bass_kernel_spmd

F16 = mybir.dt.float16
F32 = mybir.dt.float32
I16 = mybir.dt.int16
AF = mybir.ActivationFunctionType
ALU = mybir.AluOpType

N_NODES = 30000
FIN = 25
FE = 14
D = 256
N_GRAPHS = 512
N_CORES = 8
GPC = N_GRAPHS // N_CORES      # graphs per core
SLOT = 128                     # grid slots per graph
GRID = GPC * SLOT              # grid rows per core
NCH = GRID // 128              # grid chunks per core (== GPC)
NPAD = ((N_NODES + 511) // 512) * 512
GCHUNKS = 32                   # edge chunks per input-stream DMA


def _f16(a):
    return np.ascontiguousarray(np.asarray(a, np.float32).astype(np.float16))


def _host_prep(inp):
    """Pure index/layout/dtype work: build per-core input maps."""
    x = np.asarray(inp["x"], np.float32)
    ea = np.asarray(inp["edge_attr"], np.float32)
    ei = np.asarray(inp["edge_index"])
    batch = np.asarray(inp["batch"]).astype(np.int64)
    src_all = np.asarray(ei[0], np.int64)
    dst_all = np.asarray(ei[1], np.int64)

    counts = np.bincount(batch, minlength=N_GRAPHS)
    assert counts.max() <= SLOT, f"graph larger than SLOT: {counts.max()}"
    starts = np.zeros(N_GRAPHS + 1, np.int64)
    np.cumsum(counts, out=starts[1:])

    gslot = (batch % GPC) * SLOT + (np.arange(N_NODES) - starts[batch])
    dst_core = batch[dst_all] // GPC
    dst_gslot = gslot[dst_all]

    epg = np.bincount(batch[dst_all], minlength=N_GRAPHS)
    EPC = max(1, int(np.ceil(epg.max() / 128.0)))
    NEC = NCH * EPC                    # edge chunks per core (64*EPC, %16==0)
    EP = NEC * 128

    W0 = np.asarray(inp["W0"], np.float32); b0 = np.asarray(inp["b0"], np.float32)
    Wm1 = np.asarray(inp["Wm1"], np.float32); bm1 = np.asarray(inp["bm1"], np.float32)
    Wm2 = np.asarray(inp["Wm2"], np.float32); bm2 = np.asarray(inp["bm2"], np.float32)
    Wr = np.asarray(inp["Wr"], np.float32); br = np.asarray(inp["br"], np.float32)
    Wih = np.asarray(inp["Wih"], np.float32); Whh = np.asarray(inp["Whh"], np.float32)
    bl = np.asarray(inp["bl"], np.float32)
    W1 = np.asarray(inp["W1"], np.float32); b1 = np.asarray(inp["b1"], np.float32)
    W2 = np.asarray(inp["W2"], np.float32); b2 = np.asarray(inp["b2"], np.float32)

    W0c = _f16(np.concatenate([W0, b0[None, :]], 0))            # [26, 256]
    Wm1h = _f16(Wm1[:D])
    Wm1ec = _f16(np.concatenate([Wm1[D:], bm1[None, :]], 0))    # [15, 256]
    Wih_s = Wih.copy(); Wih_s[:D] *= 0.5                        # h state kept as 2h
    W1_s = W1.copy(); W1_s[:D] *= 0.5
    W1p = np.zeros((128, 4, 2, 128), np.float16)
    for kk in range(4):
        for m in range(2):
            W1p[:, kk, m, :] = _f16(W1_s[kk * 128:(kk + 1) * 128,
                                         m * 128:(m + 1) * 128])
    b1c = np.zeros((128, 2), np.float32)
    b1c[:, 0] = b1[:128]; b1c[:, 1] = b1[128:]
    W2s = np.zeros((128, 2), np.float16)
    W2s[:, 0] = _f16(W2[:128, 0]); W2s[:, 1] = _f16(W2[128:, 0])

    shared = dict(
        W0c=W0c,
        Wm1h_hi=_f16(Wm1h[:128]), Wm1h_lo=_f16(Wm1h[128:]),
        Wm1ec=Wm1ec,
        Wm2_hi=_f16(Wm2[:128]), Wm2_lo=_f16(Wm2[128:]),
        Wr_hi=_f16(Wr[:128]), Wr_lo=_f16(Wr[128:]),
        bmbr=_f16(np.stack([bm2, br], 0)),
        Wih=np.ascontiguousarray(_f16(Wih_s).reshape(4, 128, 1024).transpose(1, 0, 2)),
        Whh=np.ascontiguousarray(_f16(Whh * 0.5).reshape(2, 128, 1024).transpose(1, 0, 2)),
        blr=_f16(bl[None, :]),
        W1p=W1p, b1c=b1c, W2s=W2s, b2t=_f16(b2.reshape(1, 1)),
        ones1=np.ones((1, 64), np.float16),
        iota_row=np.tile(np.arange(128, dtype=np.float16)[None, :], (128, 1)),
        ident=np.eye(128, dtype=np.float16),
    )

    in_maps = []
    for k in range(N_CORES):
        g0 = k * GPC
        ns, ne = int(starts[g0]), int(starts[g0 + GPC])
        nodes = np.arange(ns, ne)
        gs = gslot[nodes]
        gr = batch[nodes] - g0

        xTg = np.zeros((FIN + 1, GRID), np.float16)
        xTg[:FIN, gs] = _f16(x[nodes].T)
        xTg[FIN, :] = 1.0

        Gp = np.zeros((128, NCH * GPC), np.float16)
        Gp[gs % 128, (gs // 128) * GPC + gr] = 1.0
        GTp = np.zeros((64, GRID), np.float16)
        GTp[gr, gs] = 1.0

        m = dst_core == k
        e_src = src_all[m]; e_slot = dst_gslot[m]; e_ea = ea[m]
        e_chunk = e_slot // 128
        order = np.argsort(e_chunk, kind="stable")
        e_src, e_slot, e_ea = e_src[order], e_slot[order], e_ea[order]
        e_chunk = e_chunk[order]

        deg = np.zeros(GRID, np.float32)
        np.add.at(deg, e_slot, 1.0)
        degones = np.zeros((2, GRID), np.float16)
        degones[0] = deg.astype(np.float16); degones[1] = 1.0

        srcp = np.zeros(EP, np.int64)
        colp = np.full(EP, 255.0, np.float32)
        eap = np.zeros((EP, FE + 1), np.float16)
        cstart = np.searchsorted(e_chunk, np.arange(NCH + 1))
        for c in range(NCH):
            a, b = int(cstart[c]), int(cstart[c + 1])
            n_e = b - a
            assert n_e <= EPC * 128
            o = c * EPC * 128
            srcp[o:o + n_e] = e_src[a:b]
            colp[o:o + n_e] = (e_slot[a:b] % 128).astype(np.float32)
            eap[o:o + n_e, :FE] = _f16(e_ea[a:b])
            eap[o:o + n_e, FE] = 1.0

        xgT = np.empty((FIN + 1, EP), np.float16)
        xgT[:FIN] = _f16(x[srcp].T)
        xgT[FIN] = 1.0
        xgTc = np.ascontiguousarray(
            xgT.reshape(FIN + 1, NEC, 128).transpose(0, 1, 2).reshape(FIN + 1, EP))
        dstcol = np.ascontiguousarray(colp.astype(np.float32).reshape(-1, 128).T)
        eaT = np.ascontiguousarray(
            eap.reshape(NEC, 128, FE + 1).transpose(2, 0, 1).reshape(FE + 1, EP))

        im = dict(shared)
        im.update(xTg=xTg, Gp=Gp, GTp=GTp, degones=degones,
                  xgT=xgTc, dstcol=dstcol, eaT=eaT)
        in_maps.append(im)

    return in_maps, EPC, NEC


def _build(nc, tc, EPC, NEC):
    """Emit one core's program (identical across cores; data differs)."""
    NZC = NPAD // 512
    NGG = GRID // 512
    NGROUP = NEC // GCHUNKS

    def dram_in(name, shape, dt):
        return nc.dram_tensor(name, list(shape), dt, kind="ExternalInput")

    xTg_d = dram_in("xTg", (FIN + 1, GRID), F16)
    xgT_d = dram_in("xgT", (FIN + 1, NEC * 128), F16)
    W0c_d = dram_in("W0c", (FIN + 1, D), F16)
    Wm1h_hi_d = dram_in("Wm1h_hi", (128, D), F16)
    Wm1h_lo_d = dram_in("Wm1h_lo", (128, D), F16)
    Wm1ec_d = dram_in("Wm1ec", (FE + 1, D), F16)
    Wm2_hi_d = dram_in("Wm2_hi", (128, D), F16)
    Wm2_lo_d = dram_in("Wm2_lo", (128, D), F16)
    Wr_hi_d = dram_in("Wr_hi", (128, D), F16)
    Wr_lo_d = dram_in("Wr_lo", (128, D), F16)
    bmbr_d = dram_in("bmbr", (2, D), F16)
    Wih_d = dram_in("Wih", (128, 4, 1024), F16)
    Whh_d = dram_in("Whh", (128, 2, 1024), F16)
    blr_d = dram_in("blr", (1, 1024), F16)
    W1p_d = dram_in("W1p", (128, 4, 2, 128), F16)
    b1c_d = dram_in("b1c", (128, 2), F32)
    W2s_d = dram_in("W2s", (128, 2), F16)
    b2t_d = dram_in("b2t", (1, 1), F16)
    ones1_d = dram_in("ones1", (1, 64), F16)
    iota_d = dram_in("iota_row", (128, 128), F16)
    ident_d = dram_in("ident", (128, 128), F16)
    Gp_d = dram_in("Gp", (128, NCH * GPC), F16)
    GTp_d = dram_in("GTp", (64, GRID), F16)
    degones_d = dram_in("degones", (2, GRID), F16)
    eaT_d = dram_in("eaT", (FE + 1, NEC * 128), F16)
    dstcol_d = dram_in("dstcol", (128, NEC), F32)

    y_d = nc.dram_tensor("y", [64, 1], F32, kind="ExternalOutput")

    def sb(name, shape, dt):
        return nc.alloc_sbuf_tensor(name, list(shape), dt).ap()

    s_w0 = sb("s_w0", (FIN + 1, D), F16)
    s_wm1hi = sb("s_wm1hi", (128, D), F16)
    s_wm1lo = sb("s_wm1lo", (128, D), F16)
    s_wm1ec = sb("s_wm1ec", (FE + 1, D), F16)
    s_wm2hi = sb("s_wm2hi", (128, D), F16)
    s_wm2lo = sb("s_wm2lo", (128, D), F16)
    s_wrhi = sb("s_wrhi", (128, D), F16)
    s_wrlo = sb("s_wrlo", (128, D), F16)
    s_bmbr = sb("s_bmbr", (2, D), F16)
    s_wih = sb("s_wih", (128, 4, 1024), F16)
    s_whh = sb("s_whh", (128, 2, 1024), F16)
    s_blr = sb("s_blr", (1, 1024), F16)
    s_w1 = sb("s_w1", (128, 4, 2, 128), F16)
    s_b1 = sb("s_b1", (128, 2), F32)
    s_w2 = sb("s_w2", (128, 2), F16)
    s_b2 = sb("s_b2", (1, 1), F16)
    s_ones1 = sb("s_ones1", (1, 64), F16)
    s_iota = sb("s_iota", (128, 128), F16)
    s_ident = sb("s_ident", (128, 128), F16)
    s_G = sb("s_G", (128, NCH, GPC), F16)
    s_GT = sb("s_GT", (64, GRID), F16)
    s_dego = sb("s_dego", (2, GRID), F16)
    s_dstcol = sb("s_dstcol", (128, NEC), F32)
    s_h0g_hi = sb("s_h0g_hi", (128, GRID), F16)
    s_h0g_lo = sb("s_h0g_lo", (128, GRID), F16)
    s_out = sb("s_out", (128, NCH, D + 1), F16)
    s_e = sb("s_e", (128, NCH), F32)
    s_wt = sb("s_wt", (128, NCH), F32)
    s_hT = [sb(f"s_hT{i}", (128, 64), F16) for i in range(2)]
    s_rT = [sb(f"s_rT{i}", (128, 64), F16) for i in range(2)]
    s_cu = sb("s_cu", (64, D), F32)
    s_hh16 = sb("s_hh16", (64, D), F16)
    s_y1 = [sb(f"s_y1_{i}", (128, 64), F16) for i in range(2)]
    s_yo = sb("s_yo", (64, 1), F32)

    dma = nc.sync.dma_start
    V, A, T, GP = nc.vector, nc.scalar, nc.tensor, nc.gpsimd

    for s, d in [(s_w0, W0c_d), (s_wm1hi, Wm1h_hi_d), (s_wm1lo, Wm1h_lo_d),
                 (s_wm1ec, Wm1ec_d), (s_wm2hi, Wm2_hi_d), (s_wm2lo, Wm2_lo_d),
                 (s_wrhi, Wr_hi_d), (s_wrlo, Wr_lo_d), (s_bmbr, bmbr_d),
                 (s_wih, Wih_d), (s_whh, Whh_d), (s_blr, blr_d),
                 (s_w1, W1p_d), (s_b1, b1c_d), (s_w2, W2s_d), (s_b2, b2t_d),
                 (s_ones1, ones1_d), (s_iota, iota_d), (s_ident, ident_d),
                 (s_GT, GTp_d), (s_dego, degones_d),
                 (s_dstcol, dstcol_d)]:
        dma(s[:], d[:])
    dma(s_G[:], Gp_d[:].rearrange("p (c g) -> p c g", g=GPC))

    V.memset(s_out[:, :, D:D + 1], 1.0)
    for t_ in (*s_hT, *s_rT):
        V.memset(t_[:], 0.0)
    V.memset(s_cu[:], 0.0)

    # ============ P1: grid h0T (resident, feeds the root update) ============
    with tc.tile_pool(name="p1ps", bufs=2, space="PSUM") as pp, \
         tc.tile_pool(name="p1sb", bufs=3) as ps:
        for cg in range(NGG):
            sl = slice(cg * 512, (cg + 1) * 512)
            xin = ps.tile([FIN + 1, 512], F16, tag="xin")
            dma(xin[:], xTg_d[:, sl])
            ph = pp.tile([128, 512], F32, tag="h0hi")
            pl = pp.tile([128, 512], F32, tag="h0lo")
            T.matmul(ph[:], s_w0[:, 0:128], xin[:])
            T.matmul(pl[:], s_w0[:, 128:256], xin[:])
            A.activation(s_h0g_hi[:, sl], ph[:], AF.Relu)
            V.tensor_relu(s_h0g_lo[:, sl], pl[:])

    # ============ P2: edge pipeline + segment sum + root update =============
    with tc.tile_pool(name="p2zg", bufs=3) as pzg, \
         tc.tile_pool(name="p2ea", bufs=3) as pea, \
         tc.tile_pool(name="p2sb", bufs=5) as ps2, \
         tc.tile_pool(name="p2ags", bufs=2) as pag, \
         tc.tile_pool(name="p2eaw", bufs=2, space="PSUM") as peaw, \
         tc.tile_pool(name="p2tp", bufs=3, space="PSUM") as ptp, \
         tc.tile_pool(name="p2agg", bufs=1, space="PSUM") as pagg, \
         tc.tile_pool(name="p2out", bufs=1, space="PSUM") as pout:
        for g in range(NGROUP):
            xgt = pzg.tile([FIN + 1, GCHUNKS * 128], F16, tag="xgt")
            dma(xgt[:], xgT_d[:, g * GCHUNKS * 128:(g + 1) * GCHUNKS * 128])
            eat = pea.tile([FE + 1, GCHUNKS * 128], F16, tag="eat")
            dma(eat[:], eaT_d[:, g * GCHUNKS * 128:(g + 1) * GCHUNKS * 128])
            for j2 in range(GCHUNKS // 2):
                psl = slice(j2 * 256, (j2 + 1) * 256)
                tp = ptp.tile([128, 2, 256], F32, tag="tp")
                T.matmul(tp[:, 0, :], s_w0[:, 0:128], xgt[:, psl])
                T.matmul(tp[:, 1, :], s_w0[:, 128:256], xgt[:, psl])
                t16 = ps2.tile([128, 2, 256], F16, tag="t16")
                A.activation(t16[:, 0, :], tp[:, 0, :], AF.Relu)
                V.tensor_relu(t16[:, 1, :], tp[:, 1, :])
                for h in range(2):
                    j = j2 * 2 + h
                    i = g * GCHUNKS + j
                    c, jj = i // EPC, i % EPC
                    esl = slice(j * 128, (j + 1) * 128)
                    hsl = slice(h * 128, (h + 1) * 128)
                    S = ps2.tile([128, 128], F16, tag="S")
                    V.tensor_scalar(S[:], s_iota[:], s_dstcol[:, i:i + 1], None,
                                    op0=ALU.is_equal)
                    pe_ = peaw.tile([128, D], F32, tag="eaw")
                    T.matmul(pe_[:], eat[:, esl], s_wm1ec[:],
                             start=True, stop=False)
                    T.matmul(pe_[:], t16[:, 0, hsl], s_wm1hi[:],
                             start=False, stop=False)
                    T.matmul(pe_[:], t16[:, 1, hsl], s_wm1lo[:],
                             start=False, stop=True)
                    m1 = ps2.tile([128, D], F16, tag="m1")
                    A.activation(m1[:], pe_[:], AF.Relu)
                    if jj == 0:
                        agh = pagg.tile([128, 128], F32, tag="agh")
                        agl = pagg.tile([128, 128], F32, tag="agl")
                    T.matmul(agh[:], m1[:, 0:128], S[:],
                             start=(jj == 0), stop=(jj == EPC - 1))
                    T.matmul(agl[:], m1[:, 128:256], S[:],
                             start=(jj == 0), stop=(jj == EPC - 1))
                    if jj == EPC - 1:
                        ags = pag.tile([128, 2, 128], F16, tag="ags")
                        V.tensor_copy(ags[:, 0, :], agh[:])
                        A.activation(ags[:, 1, :], agl[:], AF.Copy)
                        po = pout.tile([128, D], F32, tag="po")
                        csl = slice(c * 128, (c + 1) * 128)
                        T.matmul(po[:], ags[:, 0, :], s_wm2hi[:], start=True, stop=False)
                        T.matmul(po[:], ags[:, 1, :], s_wm2lo[:], start=False, stop=False)
                        T.matmul(po[:], s_h0g_hi[:, csl], s_wrhi[:], start=False, stop=False)
                        T.matmul(po[:], s_h0g_lo[:, csl], s_wrlo[:], start=False, stop=False)
                        T.matmul(po[:], s_dego[:, csl], s_bmbr[:], start=False, stop=True)
                        A.activation(s_out[:, c, 0:D], po[:], AF.Relu)

    # ============ P3: Set2Set (3 steps) + readout ===========================
    with tc.tile_pool(name="p3ps", bufs=2, space="PSUM") as pp3, \
         tc.tile_pool(name="p3p1", bufs=1, space="PSUM") as pq3, \
         tc.tile_pool(name="p3g", bufs=1, space="PSUM") as pg3, \
         tc.tile_pool(name="p3sb", bufs=2) as ps3:
        for step in range(S2S_STEPS := 3):
            g0p = pg3.tile([64, 512], F32, tag="g0")
            g1p = pg3.tile([64, 512], F32, tag="g1")
            for half, gp in ((0, g0p), (1, g1p)):
                nsl = slice(half * 512, (half + 1) * 512)
                T.matmul(gp[:], s_ones1[:], s_blr[:, nsl], start=True, stop=False)
                for kk in range(4):
                    lhs = (s_hT + s_rT)[kk]
                    T.matmul(gp[:], lhs[:], s_wih[:, kk, nsl],
                             start=False, stop=False)
                for kk in range(2):
                    T.matmul(gp[:], s_hT[kk][:], s_whh[:, kk, nsl],
                             start=False, stop=(kk == 1))
            ti = ps3.tile([64, D], F32, tag="ti")
            tf = ps3.tile([64, D], F32, tag="tf")
            tg = ps3.tile([64, D], F32, tag="tg")
            to = ps3.tile([64, D], F32, tag="to")
            A.activation(ti[:], g0p[:, 0:256], AF.Tanh, scale=0.5)
            A.activation(tf[:], g0p[:, 256:512], AF.Tanh, scale=0.5)
            A.activation(tg[:], g1p[:, 0:256], AF.Tanh)
            A.activation(to[:], g1p[:, 256:512], AF.Tanh, scale=0.5)
            a2 = ps3.tile([64, D], F32, tag="a2")
            bv = ps3.tile([64, D], F32, tag="bv")
            V.scalar_tensor_tensor(a2[:], tf[:], 1.0, s_cu[:], ALU.add, ALU.mult)
            V.scalar_tensor_tensor(bv[:], ti[:], 1.0, tg[:], ALU.add, ALU.mult)
            V.scalar_tensor_tensor(s_cu[:], a2[:], 0.5, bv[:], ALU.mult, ALU.add)
            th = ps3.tile([64, D], F32, tag="th")
            A.activation(th[:], s_cu[:], AF.Tanh, scale=0.5)
            hh = ps3.tile([64, D], F32, tag="hh")
            V.scalar_tensor_tensor(hh[:], to[:], 1.0, th[:], ALU.add, ALU.mult)
            A.activation(s_hh16[:], hh[:], AF.Copy)
            for mth in range(2):
                ptr = pq3.tile([128, 64], F16, tag="ptr")
                T.transpose(ptr[:], s_hh16[:, mth * 128:(mth + 1) * 128],
                            s_ident[0:64, 0:64])
                V.tensor_copy(s_hT[mth][:], ptr[:])
            # attention: e, w=exp(e), r = (sum w*out)/(sum w)
            for c in range(NCH):
                csl = slice(c * 128, (c + 1) * 128)
                hb = pp3.tile([128, D], F32, tag="hb")
                T.matmul(hb[:], s_GT[:, csl], s_hh16[:])
                scr = ps3.tile([128, D], F32, tag="scr")
                V.scalar_tensor_tensor(scr[:], s_out[:, c, 0:D], 0.5, hb[:],
                                       ALU.mult, ALU.mult,
                                       accum_out=s_e[:, c:c + 1])
            A.activation(s_wt[:], s_e[:], AF.Exp)
            rw = pg3.tile([64, 257], F32, tag="rw")
            for c in range(NCH):
                gw = ps3.tile([128, 64], F16, tag="gw")
                V.tensor_scalar(gw[:], s_G[:, c, :], s_wt[:, c:c + 1], None,
                                op0=ALU.mult)
                T.matmul(rw[:], gw[:], s_out[:, c, :],
                         start=(c == 0), stop=(c == NCH - 1))
            rr = ps3.tile([64, 1], F32, tag="rr")
            V.reciprocal(rr[:], rw[:, 256:257])
            rf = ps3.tile([64, D], F16, tag="rf")
            V.tensor_scalar(rf[:], rw[:, 0:256], rr[:], None, op0=ALU.mult)
            for mth in range(2):
                ptr = pq3.tile([128, 64], F16, tag="ptr")
                T.transpose(ptr[:], rf[:, mth * 128:(mth + 1) * 128],
                            s_ident[0:64, 0:64])
                V.tensor_copy(s_rT[mth][:], ptr[:])
        # readout
        for mth in range(2):
            yp = pq3.tile([128, 64], F32, tag="yp")
            for kk in range(4):
                T.matmul(yp[:], s_w1[:, kk, mth, :], (s_hT + s_rT)[kk][:],
                         start=(kk == 0), stop=(kk == 3))
            A.activation(s_y1[mth][:], yp[:], AF.Relu, bias=s_b1[:, mth:mth + 1])
        ypo = pq3.tile([64, 1], F32, tag="ypo")
        T.matmul(ypo[:], s_y1[0][:], s_w2[:, 0:1], start=True, stop=False)
        T.matmul(ypo[:], s_y1[1][:], s_w2[:, 1:2], start=False, stop=False)
        T.matmul(ypo[:], s_ones1[:], s_b2[:], start=False, stop=True)
        V.tensor_copy(s_yo[:], ypo[:])
        dma(y_d[:], s_yo[:])


_CACHE = {}


def _get_compiled(EPC, NEC):
    key = (EPC, NEC)
    if key not in _CACHE:
        nc = bacc.Bacc("TRN2", target_bir_lowering=False, debug=False,
                       num_devices=N_CORES)
        with tile.TileContext(nc) as tc:
            _build(nc, tc, EPC, NEC)
        nc.compile()
        _CACHE[key] = nc
    return _CACHE[key]


def kernel(**inputs) -> np.ndarray:
    in_maps, EPC, NEC = _host_prep(inputs)
    nc = _get_compiled(EPC, NEC)
    res = run_bass_kernel_spmd(nc, in_maps, list(range(N_CORES)))
    y = np.concatenate([res.results[k]["y"].reshape(-1) for k in range(N_CORES)])
    return y.astype(np.float32)



# revision 5
# speedup vs baseline: 1.1315x; 1.1315x over previous
"""Trainium2 Bass kernel for nn_DMPNN_Change_678604832935 (8-core SPMD DMPNN+Set2Set).

Sharding: each core owns 64 consecutive graphs (batch is sorted) plus all edges
whose dst node falls in those graphs — segment_sum is core-local, no collectives.

v2 layout: nodes are bin-packed (LPT on in-degree) into NB 128-slot blocks per
core, so every block has nearly equal edge load: EPC = ceil(max block in-degree
/ 128) ~= 16 with ~2% padding (vs 11-chunk per-graph padding = 42% waste in v1).
The graph<->slot mapping is absorbed by host-built one-hot matrices (G, GT, S),
so Set2Set still works on the permuted grid.  Per-edge h0 is recomputed from x
(26-row stream beats gathering 256-row h0); since segment_sum is linear, m@Wm2
folds to the node side with deg(n)*bm2 as a rank-1 correction.  The scatter
one-hot S is built on the otherwise-idle Pool (gpsimd) engine; PSUM->SBUF relu
copies alternate between Act and DVE to balance them.  Softmax uses
unnormalized exp (|e|<~8) and sigmoid is synthesized from tanh so the whole
kernel uses one ACT table set.
"""

import heapq
import os
import sys

for _p in ("/opt/trn_rl_repo", "/root/.axon_site/_ro/trn_rl_repo"):
    if os.path.isdir(_p) and _p not in sys.path:
        sys.path.append(_p)

import numpy as np

import concourse.bass as bass
import concourse.bacc as bacc
import concourse.mybir as mybir
import concourse.tile as tile
from concourse.bass_utils import run_bass_kernel_spmd

F16 = mybir.dt.float16
F32 = mybir.dt.float32
AF = mybir.ActivationFunctionType
ALU = mybir.AluOpType

N_NODES = 30000
FIN = 25
FE = 14
D = 256
N_GRAPHS = 512
N_CORES = 8
GPC = N_GRAPHS // N_CORES      # graphs per core


def _f16(a):
    return np.ascontiguousarray(np.asarray(a, np.float32).astype(np.float16))


def _binpack(ndeg, NB):
    """LPT: assign nodes (by desc degree) to the lightest non-full block.
    Returns (block_id, col_in_block) per node."""
    n = len(ndeg)
    order = np.argsort(-ndeg, kind="stable")
    blk = np.empty(n, np.int64)
    col = np.empty(n, np.int64)
    cnt = np.zeros(NB, np.int64)
    heap = [(0.0, b) for b in range(NB)]
    heapq.heapify(heap)
    for idx in order:
        while True:
            w, b = heapq.heappop(heap)
            if cnt[b] < 128:
                break
        blk[idx] = b
        col[idx] = cnt[b]
        cnt[b] += 1
        if cnt[b] < 128:
            heapq.heappush(heap, (w + float(ndeg[idx]), b))
    return blk, col


def _host_prep(inp):
    """Pure index/layout/dtype work: build per-core input maps."""
    x = np.asarray(inp["x"], np.float32)
    ea = np.asarray(inp["edge_attr"], np.float32)
    ei = np.asarray(inp["edge_index"])
    batch = np.asarray(inp["batch"]).astype(np.int64)
    src_all = np.asarray(ei[0], np.int64)
    dst_all = np.asarray(ei[1], np.int64)

    counts = np.bincount(batch, minlength=N_GRAPHS)
    starts = np.zeros(N_GRAPHS + 1, np.int64)
    np.cumsum(counts, out=starts[1:])
    deg_all = np.bincount(dst_all, minlength=N_NODES).astype(np.int64)

    core_nodes = np.array([starts[(k + 1) * GPC] - starts[k * GPC]
                           for k in range(N_CORES)])
    NB = int(np.ceil(core_nodes.max() / 128.0))
    SLOTS = NB * 128

    # per-core bin-packing first, so EPC can be chosen globally
    packs = []
    max_bdeg = 0
    for k in range(N_CORES):
        ns, ne = int(starts[k * GPC]), int(starts[(k + 1) * GPC])
        nodes = np.arange(ns, ne)
        blk, col = _binpack(deg_all[nodes], NB)
        bdeg = np.bincount(blk, weights=deg_all[nodes], minlength=NB)
        max_bdeg = max(max_bdeg, int(bdeg.max()))
        packs.append((nodes, blk, col))
    EPC = max(1, int(np.ceil(max_bdeg / 128.0)))
    if EPC % 2:
        EPC += 1                      # keep 512-edge sub-groups block-aligned
    NEC = NB * EPC
    EP = NEC * 128

    W0 = np.asarray(inp["W0"], np.float32); b0 = np.asarray(inp["b0"], np.float32)
    Wm1 = np.asarray(inp["Wm1"], np.float32); bm1 = np.asarray(inp["bm1"], np.float32)
    Wm2 = np.asarray(inp["Wm2"], np.float32); bm2 = np.asarray(inp["bm2"], np.float32)
    Wr = np.asarray(inp["Wr"], np.float32); br = np.asarray(inp["br"], np.float32)
    Wih = np.asarray(inp["Wih"], np.float32); Whh = np.asarray(inp["Whh"], np.float32)
    bl = np.asarray(inp["bl"], np.float32)
    W1 = np.asarray(inp["W1"], np.float32); b1 = np.asarray(inp["b1"], np.float32)
    W2 = np.asarray(inp["W2"], np.float32); b2 = np.asarray(inp["b2"], np.float32)

    W0c = _f16(np.concatenate([W0, b0[None, :]], 0))            # [26, 256]
    Wm1h = _f16(Wm1[:D])
    Wm1ec = _f16(np.concatenate([Wm1[D:], bm1[None, :]], 0))    # [15, 256]
    Wih_s = Wih.copy(); Wih_s[:D] *= 0.5                        # h state kept as 2h
    W1_s = W1.copy(); W1_s[:D] *= 0.5
    W1p = np.zeros((128, 4, 2, 128), np.float16)
    for kk in range(4):
        for m in range(2):
            W1p[:, kk, m, :] = _f16(W1_s[kk * 128:(kk + 1) * 128,
                                         m * 128:(m + 1) * 128])
    b1c = np.zeros((128, 2), np.float32)
    b1c[:, 0] = b1[:128]; b1c[:, 1] = b1[128:]
    W2s = np.zeros((128, 2), np.float16)
    W2s[:, 0] = _f16(W2[:128, 0]); W2s[:, 1] = _f16(W2[128:, 0])

    shared = dict(
        W0c=W0c,
        Wm1h_hi=_f16(Wm1h[:128]), Wm1h_lo=_f16(Wm1h[128:]),
        Wm1ec=Wm1ec,
        Wm2_hi=_f16(Wm2[:128]), Wm2_lo=_f16(Wm2[128:]),
        Wr_hi=_f16(Wr[:128]), Wr_lo=_f16(Wr[128:]),
        bmbr=_f16(np.stack([bm2, br], 0)),
        Wih=np.ascontiguousarray(_f16(Wih_s).reshape(4, 128, 1024).transpose(1, 0, 2)),
        Whh=np.ascontiguousarray(_f16(Whh * 0.5).reshape(2, 128, 1024).transpose(1, 0, 2)),
        blr=_f16(bl[None, :]),
        W1p=W1p, b1c=b1c, W2s=W2s, b2t=_f16(b2.reshape(1, 1)),
        ones1=np.ones((1, 64), np.float16),
        iota_row=np.tile(np.arange(128, dtype=np.float16)[None, :], (128, 1)),
        ident=np.eye(128, dtype=np.float16),
    )

    in_maps = []
    for k in range(N_CORES):
        nodes, blk, col = packs[k]
        gslot = blk * 128 + col                 # slot per core-local node
        gr = batch[nodes] - k * GPC             # graph-in-core per node

        xTg = np.zeros((FIN + 1, SLOTS), np.float16)
        xTg[:FIN, gslot] = _f16(x[nodes].T)
        xTg[FIN, :] = 1.0

        Gp = np.zeros((128, NB * GPC), np.float16)
        Gp[gslot % 128, blk * GPC + gr] = 1.0
        GTp = np.zeros((64, SLOTS), np.float16)
        GTp[gr, gslot] = 1.0

        ns = int(starts[k * GPC])
        m = (batch[dst_all] // GPC) == k
        e_src = src_all[m]
        e_slot = gslot[dst_all[m] - ns]
        e_ea = ea[m]
        e_blk = e_slot >> 7
        order = np.argsort(e_blk, kind="stable")
        e_src, e_slot, e_ea = e_src[order], e_slot[order], e_ea[order]
        e_blk = e_blk[order]

        deg = np.zeros(SLOTS, np.float32)
        np.add.at(deg, e_slot, 1.0)
        degones = np.zeros((2, SLOTS), np.float16)
        degones[0] = deg.astype(np.float16); degones[1] = 1.0

        # dense per-block edge placement: block b owns slots [b*EPC*128, ...)
        bstart = np.searchsorted(e_blk, np.arange(NB + 1))
        pos = (e_blk * EPC * 128) + (np.arange(len(e_src)) - bstart[e_blk])
        assert (np.arange(len(e_src)) - bstart[e_blk]).max() < EPC * 128

        srcp = np.zeros(EP, np.int64)
        colp = np.full(EP, 255.0, np.float32)
        eap = np.zeros((EP, FE + 1), np.float16)
        srcp[pos] = e_src
        colp[pos] = (e_slot % 128).astype(np.float32)
        eap[pos, :FE] = _f16(e_ea)
        eap[pos, FE] = 1.0

        xgT = np.empty((FIN + 1, EP), np.float16)
        xgT[:FIN] = _f16(x[srcp].T)
        xgT[FIN] = 1.0
        dstcol = np.ascontiguousarray(colp.reshape(-1, 128).T)   # [128, NEC]
        eaT = np.ascontiguousarray(
            eap.reshape(NEC, 128, FE + 1).transpose(2, 0, 1).reshape(FE + 1, EP))

        im = dict(shared)
        im.update(xTg=xTg, Gp=Gp, GTp=GTp, degones=degones,
                  xgT=xgT, dstcol=dstcol, eaT=eaT)
        in_maps.append(im)

    return in_maps, NB, EPC


def _build(nc, tc, NB, EPC):
    """Emit one core's program (identical across cores; data differs)."""
    NEC = NB * EPC
    SLOTS = NB * 128
    NGG = 8                                  # h0-grid column groups
    GW = SLOTS // NGG                        # group width (<=512 for one bank)
    assert NGG * GW == SLOTS and GW <= 512

    def dram_in(name, shape, dt):
        return nc.dram_tensor(name, list(shape), dt, kind="ExternalInput")

    xTg_d = dram_in("xTg", (FIN + 1, SLOTS), F16)
    xgT_d = dram_in("xgT", (FIN + 1, NEC * 128), F16)
    W0c_d = dram_in("W0c", (FIN + 1, D), F16)
    Wm1h_hi_d = dram_in("Wm1h_hi", (128, D), F16)
    Wm1h_lo_d = dram_in("Wm1h_lo", (128, D), F16)
    Wm1ec_d = dram_in("Wm1ec", (FE + 1, D), F16)
    Wm2_hi_d = dram_in("Wm2_hi", (128, D), F16)
    Wm2_lo_d = dram_in("Wm2_lo", (128, D), F16)
    Wr_hi_d = dram_in("Wr_hi", (128, D), F16)
    Wr_lo_d = dram_in("Wr_lo", (128, D), F16)
    bmbr_d = dram_in("bmbr", (2, D), F16)
    Wih_d = dram_in("Wih", (128, 4, 1024), F16)
    Whh_d = dram_in("Whh", (128, 2, 1024), F16)
    blr_d = dram_in("blr", (1, 1024), F16)
    W1p_d = dram_in("W1p", (128, 4, 2, 128), F16)
    b1c_d = dram_in("b1c", (128, 2), F32)
    W2s_d = dram_in("W2s", (128, 2), F16)
    b2t_d = dram_in("b2t", (1, 1), F16)
    ones1_d = dram_in("ones1", (1, 64), F16)
    iota_d = dram_in("iota_row", (128, 128), F16)
    ident_d = dram_in("ident", (128, 128), F16)
    Gp_d = dram_in("Gp", (128, NB * GPC), F16)
    GTp_d = dram_in("GTp", (64, SLOTS), F16)
    degones_d = dram_in("degones", (2, SLOTS), F16)
    eaT_d = dram_in("eaT", (FE + 1, NEC * 128), F16)
    dstcol_d = dram_in("dstcol", (128, NEC), F32)

    y_d = nc.dram_tensor("y", [64, 1], F32, kind="ExternalOutput")

    def sb(name, shape, dt):
        return nc.alloc_sbuf_tensor(name, list(shape), dt).ap()

    s_w0 = sb("s_w0", (FIN + 1, D), F16)
    s_wm1hi = sb("s_wm1hi", (128, D), F16)
    s_wm1lo = sb("s_wm1lo", (128, D), F16)
    s_wm1ec = sb("s_wm1ec", (FE + 1, D), F16)
    s_wm2hi = sb("s_wm2hi", (128, D), F16)
    s_wm2lo = sb("s_wm2lo", (128, D), F16)
    s_wrhi = sb("s_wrhi", (128, D), F16)
    s_wrlo = sb("s_wrlo", (128, D), F16)
    s_bmbr = sb("s_bmbr", (2, D), F16)
    s_wih = sb("s_wih", (128, 4, 1024), F16)
    s_whh = sb("s_whh", (128, 2, 1024), F16)
    s_blr = sb("s_blr", (1, 1024), F16)
    s_w1 = sb("s_w1", (128, 4, 2, 128), F16)
    s_b1 = sb("s_b1", (128, 2), F32)
    s_w2 = sb("s_w2", (128, 2), F16)
    s_b2 = sb("s_b2", (1, 1), F16)
    s_ones1 = sb("s_ones1", (1, 64), F16)
    s_iota = sb("s_iota", (128, 128), F16)
    s_ident = sb("s_ident", (128, 128), F16)
    s_G = sb("s_G", (128, NB, GPC), F16)
    s_GT = sb("s_GT", (64, SLOTS), F16)
    s_dego = sb("s_dego", (2, SLOTS), F16)
    s_dstcol = sb("s_dstcol", (128, NEC), F32)
    s_h0g_hi = sb("s_h0g_hi", (128, SLOTS), F16)
    s_h0g_lo = sb("s_h0g_lo", (128, SLOTS), F16)
    s_out = sb("s_out", (128, NB, D + 1), F16)
    s_e = sb("s_e", (128, NB), F32)
    s_wt = sb("s_wt", (128, NB), F32)
    s_hT = [sb(f"s_hT{i}", (128, 64), F16) for i in range(2)]
    s_rT = [sb(f"s_rT{i}", (128, 64), F16) for i in range(2)]
    s_cu = sb("s_cu", (64, D), F32)
    s_hh16 = sb("s_hh16", (64, D), F16)
    s_y1 = [sb(f"s_y1_{i}", (128, 64), F16) for i in range(2)]
    s_yo = sb("s_yo", (64, 1), F32)

    dma = nc.sync.dma_start
    V, A, T, GP = nc.vector, nc.scalar, nc.tensor, nc.gpsimd

    for s, d in [(s_w0, W0c_d), (s_wm1hi, Wm1h_hi_d), (s_wm1lo, Wm1h_lo_d),
                 (s_wm1ec, Wm1ec_d), (s_wm2hi, Wm2_hi_d), (s_wm2lo, Wm2_lo_d),
                 (s_wrhi, Wr_hi_d), (s_wrlo, Wr_lo_d), (s_bmbr, bmbr_d),
                 (s_wih, Wih_d), (s_whh, Whh_d), (s_blr, blr_d),
                 (s_w1, W1p_d), (s_b1, b1c_d), (s_w2, W2s_d), (s_b2, b2t_d),
                 (s_ones1, ones1_d), (s_iota, iota_d), (s_ident, ident_d),
                 (s_GT, GTp_d), (s_dego, degones_d),
                 (s_dstcol, dstcol_d)]:
        dma(s[:], d[:])
    dma(s_G[:], Gp_d[:].rearrange("p (c g) -> p c g", g=GPC))

    V.memset(s_out[:, :, D:D + 1], 1.0)
    for t_ in (*s_hT, *s_rT):
        V.memset(t_[:], 0.0)
    V.memset(s_cu[:], 0.0)

    # ============ P1: grid h0T (resident, feeds the root update) ============
    with tc.tile_pool(name="p1ps", bufs=2, space="PSUM") as pp, \
         tc.tile_pool(name="p1sb", bufs=3) as ps:
        for cg in range(NGG):
            sl = slice(cg * GW, (cg + 1) * GW)
            xin = ps.tile([FIN + 1, GW], F16, tag="xin")
            dma(xin[:], xTg_d[:, sl])
            ph = pp.tile([128, GW], F32, tag="h0hi")
            pl = pp.tile([128, GW], F32, tag="h0lo")
            T.matmul(ph[:], s_w0[:, 0:128], xin[:])
            T.matmul(pl[:], s_w0[:, 128:256], xin[:])
            A.activation(s_h0g_hi[:, sl], ph[:], AF.Relu)
            V.tensor_relu(s_h0g_lo[:, sl], pl[:])

    # ============ P2: edge pipeline + segment sum + root update =============
    # one block = EPC chunks of 128 edges, all scattering into 128 dst slots
    EB = EPC * 128                       # edges per block
    with tc.tile_pool(name="p2zg", bufs=3) as pzg, \
         tc.tile_pool(name="p2ea", bufs=3) as pea, \
         tc.tile_pool(name="p2sb", bufs=5) as ps2, \
         tc.tile_pool(name="p2ags", bufs=2) as pag, \
         tc.tile_pool(name="p2eaw", bufs=2, space="PSUM") as peaw, \
         tc.tile_pool(name="p2tp", bufs=2, space="PSUM") as ptp, \
         tc.tile_pool(name="p2agg", bufs=1, space="PSUM") as pagg, \
         tc.tile_pool(name="p2out", bufs=2, space="PSUM") as pout:
        for b in range(NB):
            xgt = pzg.tile([FIN + 1, EB], F16, tag="xgt")
            dma(xgt[:], xgT_d[:, b * EB:(b + 1) * EB])
            eat = pea.tile([FE + 1, EB], F16, tag="eat")
            dma(eat[:], eaT_d[:, b * EB:(b + 1) * EB])
            agh = pagg.tile([128, 128], F32, tag="agh")
            agl = pagg.tile([128, 128], F32, tag="agl")
            for j2 in range(EPC // 2):
                psl = slice(j2 * 256, (j2 + 1) * 256)
                tp = ptp.tile([128, 2, 256], F32, tag="tp")
                T.matmul(tp[:, 0, :], s_w0[:, 0:128], xgt[:, psl])
                T.matmul(tp[:, 1, :], s_w0[:, 128:256], xgt[:, psl])
                t16 = ps2.tile([128, 2, 256], F16, tag="t16")
                A.activation(t16[:, 0, :], tp[:, 0, :], AF.Relu)
                V.tensor_relu(t16[:, 1, :], tp[:, 1, :])
                for h in range(2):
                    j = j2 * 2 + h
                    i = b * EPC + j
                    esl = slice(j * 128, (j + 1) * 128)
                    hsl = slice(h * 128, (h + 1) * 128)
                    S = ps2.tile([128, 128], F16, tag="S")
                    GP.tensor_scalar(S[:], s_iota[:], s_dstcol[:, i:i + 1], None,
                                     op0=ALU.is_equal)
                    pe_ = peaw.tile([128, D], F32, tag="eaw")
                    T.matmul(pe_[:], eat[:, esl], s_wm1ec[:],
                             start=True, stop=False)
                    T.matmul(pe_[:], t16[:, 0, hsl], s_wm1hi[:],
                             start=False, stop=False)
                    T.matmul(pe_[:], t16[:, 1, hsl], s_wm1lo[:],
                             start=False, stop=True)
                    m1 = ps2.tile([128, D], F16, tag="m1")
                    if j % 2 == 0:
                        A.activation(m1[:], pe_[:], AF.Relu)
                    else:
                        V.tensor_relu(m1[:], pe_[:])
                    T.matmul(agh[:], m1[:, 0:128], S[:],
                             start=(j == 0), stop=(j == EPC - 1))
                    T.matmul(agl[:], m1[:, 128:256], S[:],
                             start=(j == 0), stop=(j == EPC - 1))
            # block tail: fold Wm2, root update, relu
            ags = pag.tile([128, 2, 128], F16, tag="ags")
            V.tensor_copy(ags[:, 0, :], agh[:])
            A.activation(ags[:, 1, :], agl[:], AF.Copy)
            po = pout.tile([128, D], F32, tag="po")
            csl = slice(b * 128, (b + 1) * 128)
            T.matmul(po[:], ags[:, 0, :], s_wm2hi[:], start=True, stop=False)
            T.matmul(po[:], ags[:, 1, :], s_wm2lo[:], start=False, stop=False)
            T.matmul(po[:], s_h0g_hi[:, csl], s_wrhi[:], start=False, stop=False)
            T.matmul(po[:], s_h0g_lo[:, csl], s_wrlo[:], start=False, stop=False)
            T.matmul(po[:], s_dego[:, csl], s_bmbr[:], start=False, stop=True)
            A.activation(s_out[:, b, 0:D], po[:], AF.Relu)

    # ============ P3: Set2Set (3 steps) + readout ===========================
    with tc.tile_pool(name="p3ps", bufs=2, space="PSUM") as pp3, \
         tc.tile_pool(name="p3p1", bufs=1, space="PSUM") as pq3, \
         tc.tile_pool(name="p3g", bufs=1, space="PSUM") as pg3, \
         tc.tile_pool(name="p3sb", bufs=2) as ps3:
        for step in range(S2S_STEPS := 3):
            g0p = pg3.tile([64, 512], F32, tag="g0")
            g1p = pg3.tile([64, 512], F32, tag="g1")
            for half, gp in ((0, g0p), (1, g1p)):
                nsl = slice(half * 512, (half + 1) * 512)
                T.matmul(gp[:], s_ones1[:], s_blr[:, nsl], start=True, stop=False)
                for kk in range(4):
                    lhs = (s_hT + s_rT)[kk]
                    T.matmul(gp[:], lhs[:], s_wih[:, kk, nsl],
                             start=False, stop=False)
                for kk in range(2):
                    T.matmul(gp[:], s_hT[kk][:], s_whh[:, kk, nsl],
                             start=False, stop=(kk == 1))
            ti = ps3.tile([64, D], F32, tag="ti")
            tf = ps3.tile([64, D], F32, tag="tf")
            tg = ps3.tile([64, D], F32, tag="tg")
            to = ps3.tile([64, D], F32, tag="to")
            A.activation(ti[:], g0p[:, 0:256], AF.Tanh, scale=0.5)
            A.activation(tf[:], g0p[:, 256:512], AF.Tanh, scale=0.5)
            A.activation(tg[:], g1p[:, 0:256], AF.Tanh)
            A.activation(to[:], g1p[:, 256:512], AF.Tanh, scale=0.5)
            a2 = ps3.tile([64, D], F32, tag="a2")
            bv = ps3.tile([64, D], F32, tag="bv")
            V.scalar_tensor_tensor(a2[:], tf[:], 1.0, s_cu[:], ALU.add, ALU.mult)
            V.scalar_tensor_tensor(bv[:], ti[:], 1.0, tg[:], ALU.add, ALU.mult)
            V.scalar_tensor_tensor(s_cu[:], a2[:], 0.5, bv[:], ALU.mult, ALU.add)
            th = ps3.tile([64, D], F32, tag="th")
            A.activation(th[:], s_cu[:], AF.Tanh, scale=0.5)
            hh = ps3.tile([64, D], F32, tag="hh")
            V.scalar_tensor_tensor(hh[:], to[:], 1.0, th[:], ALU.add, ALU.mult)
            A.activation(s_hh16[:], hh[:], AF.Copy)
            for mth in range(2):
                ptr = pq3.tile([128, 64], F16, tag="ptr")
                T.transpose(ptr[:], s_hh16[:, mth * 128:(mth + 1) * 128],
                            s_ident[0:64, 0:64])
                V.tensor_copy(s_hT[mth][:], ptr[:])
            # attention: e, w=exp(e), r = (sum w*out)/(sum w)
            for c in range(NB):
                csl = slice(c * 128, (c + 1) * 128)
                hb = pp3.tile([128, D], F32, tag="hb")
                T.matmul(hb[:], s_GT[:, csl], s_hh16[:])
                scr = ps3.tile([128, D], F32, tag="scr")
                V.scalar_tensor_tensor(scr[:], s_out[:, c, 0:D], 0.5, hb[:],
                                       ALU.mult, ALU.mult,
                                       accum_out=s_e[:, c:c + 1])
            A.activation(s_wt[:], s_e[:], AF.Exp)
            rw = pg3.tile([64, 257], F32, tag="rw")
            for c in range(NB):
                gw = ps3.tile([128, 64], F16, tag="gw")
                V.tensor_scalar(gw[:], s_G[:, c, :], s_wt[:, c:c + 1], None,
                                op0=ALU.mult)
                T.matmul(rw[:], gw[:], s_out[:, c, :],
                         start=(c == 0), stop=(c == NB - 1))
            rr = ps3.tile([64, 1], F32, tag="rr")
            V.reciprocal(rr[:], rw[:, 256:257])
            rf = ps3.tile([64, D], F16, tag="rf")
            V.tensor_scalar(rf[:], rw[:, 0:256], rr[:], None, op0=ALU.mult)
            for mth in range(2):
                ptr = pq3.tile([128, 64], F16, tag="ptr")
                T.transpose(ptr[:], rf[:, mth * 128:(mth + 1) * 128],
                            s_ident[0:64, 0:64])
                V.tensor_copy(s_rT[mth][:], ptr[:])
        # readout
        for mth in range(2):
            yp = pq3.tile([128, 64], F32, tag="yp")
            for kk in range(4):
                T.matmul(yp[:], s_w1[:, kk, mth, :], (s_hT + s_rT)[kk][:],
                         start=(kk == 0), stop=(kk == 3))
            A.activation(s_y1[mth][:], yp[:], AF.Relu, bias=s_b1[:, mth:mth + 1])
        ypo = pq3.tile([64, 1], F32, tag="ypo")
        T.matmul(ypo[:], s_y1[0][:], s_w2[:, 0:1], start=True, stop=False)
        T.matmul(ypo[:], s_y1[1][:], s_w2[:, 1:2], start=False, stop=False)
        T.matmul(ypo[:], s_ones1[:], s_b2[:], start=False, stop=True)
        V.tensor_copy(s_yo[:], ypo[:])
        dma(y_d[:], s_yo[:])


_CACHE = {}


def _get_compiled(NB, EPC):
    key = (NB, EPC)
    if key not in _CACHE:
        nc = bacc.Bacc("TRN2", target_bir_lowering=False, debug=False,
                       num_devices=N_CORES)
        with tile.TileContext(nc) as tc:
            _build(nc, tc, NB, EPC)
        nc.compile()
        _CACHE[key] = nc
    return _CACHE[key]


def kernel(**inputs) -> np.ndarray:
    in_maps, NB, EPC = _host_prep(inputs)
    nc = _get_compiled(NB, EPC)
    res = run_bass_kernel_spmd(nc, in_maps, list(range(N_CORES)))
    y = np.concatenate([res.results[k]["y"].reshape(-1) for k in range(N_CORES)])
    return y.astype(np.float32)


# revision 18
# speedup vs baseline: 53.9566x; 47.6850x over previous
"""Trainium2 Bass kernel for nn_DMPNN_Change_678604832935 (8-core SPMD DMPNN+Set2Set).

Sharding: each core owns 64 consecutive graphs (batch is sorted) plus all edges
whose dst node falls in those graphs — segment_sum is core-local, no collectives.

v2 layout: nodes are bin-packed (LPT on in-degree) into NB 128-slot blocks per
core, so every block has nearly equal edge load: EPC = ceil(max block in-degree
/ 128) ~= 16 with ~2% padding (vs 11-chunk per-graph padding = 42% waste in v1).
The graph<->slot mapping is absorbed by host-built one-hot matrices (G, GT, S),
so Set2Set still works on the permuted grid.  Per-edge h0 is recomputed from x
(26-row stream beats gathering 256-row h0); since segment_sum is linear, m@Wm2
folds to the node side with deg(n)*bm2 as a rank-1 correction.  The scatter
one-hot S is built on the otherwise-idle Pool (gpsimd) engine; PSUM->SBUF relu
copies alternate between Act and DVE to balance them.  Softmax uses
unnormalized exp (|e|<~8) and sigmoid is synthesized from tanh so the whole
kernel uses one ACT table set.
"""

import heapq
import os
import sys

for _p in ("/opt/trn_rl_repo", "/root/.axon_site/_ro/trn_rl_repo"):
    if os.path.isdir(_p) and _p not in sys.path:
        sys.path.append(_p)

import numpy as np

import concourse.bass as bass
import concourse.bacc as bacc
import concourse.mybir as mybir
import concourse.tile as tile
from concourse.bass_utils import run_bass_kernel_spmd

F16 = mybir.dt.float16
F32 = mybir.dt.float32
AF = mybir.ActivationFunctionType
ALU = mybir.AluOpType

N_NODES = 30000
FIN = 25
FE = 14
D = 256
N_GRAPHS = 512
N_CORES = 8
GPC = N_GRAPHS // N_CORES      # graphs per core


def _f16(a):
    return np.ascontiguousarray(np.asarray(a, np.float32).astype(np.float16))


def _binpack(ndeg, NB):
    """LPT: assign nodes (by desc degree) to the lightest non-full block.
    Returns (block_id, col_in_block) per node."""
    n = len(ndeg)
    order = np.argsort(-ndeg, kind="stable")
    blk = np.empty(n, np.int64)
    col = np.empty(n, np.int64)
    cnt = np.zeros(NB, np.int64)
    heap = [(0.0, b) for b in range(NB)]
    heapq.heapify(heap)
    for idx in order:
        while True:
            w, b = heapq.heappop(heap)
            if cnt[b] < 128:
                break
        blk[idx] = b
        col[idx] = cnt[b]
        cnt[b] += 1
        if cnt[b] < 128:
            heapq.heappush(heap, (w + float(ndeg[idx]), b))
    return blk, col


def _host_prep(inp):
    """Pure index/layout/dtype work: build per-core input maps."""
    x = np.asarray(inp["x"], np.float32)
    ea = np.asarray(inp["edge_attr"], np.float32)
    ei = np.asarray(inp["edge_index"])
    batch = np.asarray(inp["batch"]).astype(np.int64)
    src_all = np.asarray(ei[0], np.int64)
    dst_all = np.asarray(ei[1], np.int64)

    counts = np.bincount(batch, minlength=N_GRAPHS)
    starts = np.zeros(N_GRAPHS + 1, np.int64)
    np.cumsum(counts, out=starts[1:])
    deg_all = np.bincount(dst_all, minlength=N_NODES).astype(np.int64)

    core_nodes = np.array([starts[(k + 1) * GPC] - starts[k * GPC]
                           for k in range(N_CORES)])
    NB = int(np.ceil(core_nodes.max() / 128.0))
    SLOTS = NB * 128

    # per-core bin-packing first, so EPC can be chosen globally
    packs = []
    max_bdeg = 0
    for k in range(N_CORES):
        ns, ne = int(starts[k * GPC]), int(starts[(k + 1) * GPC])
        nodes = np.arange(ns, ne)
        blk, col = _binpack(deg_all[nodes], NB)
        bdeg = np.bincount(blk, weights=deg_all[nodes], minlength=NB)
        max_bdeg = max(max_bdeg, int(bdeg.max()))
        packs.append((nodes, blk, col))
    EPC = max(1, int(np.ceil(max_bdeg / 128.0)))
    if EPC % 2:
        EPC += 1                      # keep 512-edge sub-groups block-aligned
    NEC = NB * EPC
    EP = NEC * 128

    W0 = np.asarray(inp["W0"], np.float32); b0 = np.asarray(inp["b0"], np.float32)
    Wm1 = np.asarray(inp["Wm1"], np.float32); bm1 = np.asarray(inp["bm1"], np.float32)
    Wm2 = np.asarray(inp["Wm2"], np.float32); bm2 = np.asarray(inp["bm2"], np.float32)
    Wr = np.asarray(inp["Wr"], np.float32); br = np.asarray(inp["br"], np.float32)
    Wih = np.asarray(inp["Wih"], np.float32); Whh = np.asarray(inp["Whh"], np.float32)
    bl = np.asarray(inp["bl"], np.float32)
    W1 = np.asarray(inp["W1"], np.float32); b1 = np.asarray(inp["b1"], np.float32)
    W2 = np.asarray(inp["W2"], np.float32); b2 = np.asarray(inp["b2"], np.float32)

    W0c = _f16(np.concatenate([W0, b0[None, :]], 0))            # [26, 256]
    Wm1h = _f16(Wm1[:D])
    Wm1ec = _f16(np.concatenate([Wm1[D:], bm1[None, :]], 0))    # [15, 256]
    Wih_s = Wih.copy(); Wih_s[:D] *= 0.5                        # h state kept as 2h
    W1_s = W1.copy(); W1_s[:D] *= 0.5
    W1p = np.zeros((128, 4, 2, 128), np.float16)
    for kk in range(4):
        for m in range(2):
            W1p[:, kk, m, :] = _f16(W1_s[kk * 128:(kk + 1) * 128,
                                         m * 128:(m + 1) * 128])
    b1c = np.zeros((128, 2), np.float32)
    b1c[:, 0] = b1[:128]; b1c[:, 1] = b1[128:]
    W2s = np.zeros((128, 2), np.float16)
    W2s[:, 0] = _f16(W2[:128, 0]); W2s[:, 1] = _f16(W2[128:, 0])

    Wpk = np.zeros((128, 1152), np.float16)
    Wpk[:FIN + 1, 0:256] = W0c
    Wpk[:, 256:512] = _f16(Wm1h[:128])
    Wpk[:, 512:768] = _f16(Wm1h[128:])
    Wpk[:FE + 1, 768:1024] = Wm1ec
    Wpk[:, 1024:1152] = np.tile(np.arange(128, dtype=np.float16)[None, :],
                                (128, 1))
    shared = dict(
        Wpk=Wpk,
        Wm2_hi=_f16(Wm2[:128]), Wm2_lo=_f16(Wm2[128:]),
        Wr_hi=_f16(Wr[:128]), Wr_lo=_f16(Wr[128:]),
        bmbr=_f16(np.stack([bm2, br], 0)),
        Wih=np.ascontiguousarray(_f16(Wih_s).reshape(4, 128, 1024).transpose(1, 0, 2)),
        Whh=np.ascontiguousarray(_f16(Whh * 0.5).reshape(2, 128, 1024).transpose(1, 0, 2)),
        blr=_f16(bl[None, :]),
        W1p=W1p, b1c=b1c, W2s=W2s, b2t=_f16(b2.reshape(1, 1)),
        ones1=np.ones((1, 64), np.float16),
        ident=np.eye(128, dtype=np.float16),
    )

    in_maps = []
    for k in range(N_CORES):
        nodes, blk, col = packs[k]
        gslot = blk * 128 + col                 # slot per core-local node
        gr = batch[nodes] - k * GPC             # graph-in-core per node

        xTg = np.zeros((FIN + 1, SLOTS), np.float16)
        xTg[:FIN, gslot] = _f16(x[nodes].T)
        xTg[FIN, :] = 1.0

        Gp = np.zeros((128, NB * GPC), np.float16)
        Gp[gslot % 128, blk * GPC + gr] = 1.0
        GTp = np.zeros((64, SLOTS), np.float16)
        GTp[gr, gslot] = 1.0

        ns = int(starts[k * GPC])
        m = (batch[dst_all] // GPC) == k
        e_src = src_all[m]
        e_slot = gslot[dst_all[m] - ns]
        e_ea = ea[m]
        e_blk = e_slot >> 7
        order = np.argsort(e_blk, kind="stable")
        e_src, e_slot, e_ea = e_src[order], e_slot[order], e_ea[order]
        e_blk = e_blk[order]

        deg = np.zeros(SLOTS, np.float32)
        np.add.at(deg, e_slot, 1.0)
        degones = np.zeros((2, SLOTS), np.float16)
        degones[0] = deg.astype(np.float16); degones[1] = 1.0

        # dense per-block edge placement: block b owns slots [b*EPC*128, ...)
        bstart = np.searchsorted(e_blk, np.arange(NB + 1))
        pos = (e_blk * EPC * 128) + (np.arange(len(e_src)) - bstart[e_blk])
        assert (np.arange(len(e_src)) - bstart[e_blk]).max() < EPC * 128

        srcp = np.zeros(EP, np.int64)
        colp = np.full(EP, 255.0, np.float32)
        eap = np.zeros((EP, FE + 1), np.float16)
        srcp[pos] = e_src
        colp[pos] = (e_slot % 128).astype(np.float32)
        eap[pos, :FE] = _f16(e_ea)
        eap[pos, FE] = 1.0

        xgT = np.empty((FIN + 1, EP), np.float16)
        xgT[:FIN] = _f16(x[srcp].T)
        xgT[FIN] = 1.0
        dstcol = np.ascontiguousarray(colp.reshape(-1, 128).T)   # [128, NEC]
        eaT = np.ascontiguousarray(
            eap.reshape(NEC, 128, FE + 1).transpose(2, 0, 1).reshape(FE + 1, EP))

        im = dict(shared)
        im.update(xTg=xTg, Gp=Gp, GTp=GTp, degones=degones,
                  xgT=xgT, dstcol=dstcol, eaT=eaT)
        in_maps.append(im)

    return in_maps, NB, EPC


def _build(nc, tc, NB, EPC):
    """Emit one core's program (identical across cores; data differs)."""
    NEC = NB * EPC
    SLOTS = NB * 128
    NGG = 8                                  # h0-grid column groups
    GW = SLOTS // NGG                        # group width (<=512 for one bank)
    assert NGG * GW == SLOTS and GW <= 512

    def dram_in(name, shape, dt):
        return nc.dram_tensor(name, list(shape), dt, kind="ExternalInput")

    xTg_d = dram_in("xTg", (FIN + 1, SLOTS), F16)
    xgT_d = dram_in("xgT", (FIN + 1, NEC * 128), F16)
    Wpk_d = dram_in("Wpk", (128, 1152), F16)
    Wm2_hi_d = dram_in("Wm2_hi", (128, D), F16)
    Wm2_lo_d = dram_in("Wm2_lo", (128, D), F16)
    Wr_hi_d = dram_in("Wr_hi", (128, D), F16)
    Wr_lo_d = dram_in("Wr_lo", (128, D), F16)
    bmbr_d = dram_in("bmbr", (2, D), F16)
    Wih_d = dram_in("Wih", (128, 4, 1024), F16)
    Whh_d = dram_in("Whh", (128, 2, 1024), F16)
    blr_d = dram_in("blr", (1, 1024), F16)
    W1p_d = dram_in("W1p", (128, 4, 2, 128), F16)
    b1c_d = dram_in("b1c", (128, 2), F32)
    W2s_d = dram_in("W2s", (128, 2), F16)
    b2t_d = dram_in("b2t", (1, 1), F16)
    ones1_d = dram_in("ones1", (1, 64), F16)
    ident_d = dram_in("ident", (128, 128), F16)
    Gp_d = dram_in("Gp", (128, NB * GPC), F16)
    GTp_d = dram_in("GTp", (64, SLOTS), F16)
    degones_d = dram_in("degones", (2, SLOTS), F16)
    eaT_d = dram_in("eaT", (FE + 1, NEC * 128), F16)
    dstcol_d = dram_in("dstcol", (128, NEC), F32)

    y_d = nc.dram_tensor("y", [64, 1], F32, kind="ExternalOutput")

    def sb(name, shape, dt):
        return nc.alloc_sbuf_tensor(name, list(shape), dt).ap()

    s_wpk = sb("s_wpk", (128, 1152), F16)
    s_wm2hi = sb("s_wm2hi", (128, D), F16)
    s_wm2lo = sb("s_wm2lo", (128, D), F16)
    s_wrhi = sb("s_wrhi", (128, D), F16)
    s_wrlo = sb("s_wrlo", (128, D), F16)
    s_bmbr = sb("s_bmbr", (2, D), F16)
    s_wih = sb("s_wih", (128, 4, 1024), F16)
    s_whh = sb("s_whh", (128, 2, 1024), F16)
    s_blr = sb("s_blr", (1, 1024), F16)
    s_w1 = sb("s_w1", (128, 4, 2, 128), F16)
    s_b1 = sb("s_b1", (128, 2), F32)
    s_w2 = sb("s_w2", (128, 2), F16)
    s_b2 = sb("s_b2", (1, 1), F16)
    s_ones1 = sb("s_ones1", (1, 64), F16)
    s_ident = sb("s_ident", (128, 128), F16)
    s_G = sb("s_G", (128, NB, GPC), F16)
    s_GT = sb("s_GT", (64, SLOTS), F16)
    s_dego = sb("s_dego", (2, SLOTS), F16)
    s_dstcol = sb("s_dstcol", (128, NEC), F32)
    s_h0g_hi = sb("s_h0g_hi", (128, SLOTS), F16)
    s_h0g_lo = sb("s_h0g_lo", (128, SLOTS), F16)
    s_out = sb("s_out", (128, NB, D + 1), F16)
    s_e = sb("s_e", (128, NB), F32)
    s_wt = sb("s_wt", (128, NB), F32)
    s_hT = [sb(f"s_hT{i}", (128, 64), F16) for i in range(2)]
    s_rT = [sb(f"s_rT{i}", (128, 64), F16) for i in range(2)]
    s_cu = sb("s_cu", (64, D), F32)
    s_hh16 = sb("s_hh16", (64, D), F16)
    s_y1 = [sb(f"s_y1_{i}", (128, 64), F16) for i in range(2)]
    s_yo = sb("s_yo", (64, 1), F32)

    dma = nc.sync.dma_start
    V, A, T, GP = nc.vector, nc.scalar, nc.tensor, nc.gpsimd

    # only what P1 + the first P2 blocks need up-front; the rest is deferred
    # behind the first edge-stream fetches so PE isn't idle at kernel start
    dma(s_wpk[:], Wpk_d[:])
    dma(s_dstcol[:], dstcol_d[:])

    def preload_rest():
        for s, d in [(s_wm2hi, Wm2_hi_d), (s_wm2lo, Wm2_lo_d),
                     (s_wrhi, Wr_hi_d), (s_wrlo, Wr_lo_d), (s_bmbr, bmbr_d),
                     (s_dego, degones_d),
                     (s_wih, Wih_d), (s_whh, Whh_d), (s_blr, blr_d),
                     (s_w1, W1p_d), (s_b1, b1c_d), (s_w2, W2s_d),
                     (s_b2, b2t_d), (s_ones1, ones1_d), (s_ident, ident_d),
                     (s_GT, GTp_d)]:
            dma(s[:], d[:])
        dma(s_G[:], Gp_d[:].rearrange("p (c g) -> p c g", g=GPC))

    V.memset(s_out[:, :, D:D + 1], 1.0)
    for t_ in (*s_hT, *s_rT):
        V.memset(t_[:], 0.0)
    V.memset(s_cu[:], 0.0)

    # ============ P1: grid h0T (resident, feeds the root update) ============
    with tc.tile_pool(name="p1ps", bufs=2, space="PSUM") as pp, \
         tc.tile_pool(name="p1sb", bufs=3) as ps:
        HG = NGG // 2                       # grid half: one DMA, 4 matmul groups
        for half in range(2):
            xin = ps.tile([FIN + 1, HG * GW], F16, tag="xin")
            dma(xin[:], xTg_d[:, half * HG * GW:(half + 1) * HG * GW])
            for cg in range(HG):
                sl = slice(half * HG * GW + cg * GW,
                           half * HG * GW + (cg + 1) * GW)
                ph = pp.tile([128, GW], F32, tag="h0hi")
                pl = pp.tile([128, GW], F32, tag="h0lo")
                T.matmul(ph[:], s_wpk[0:FIN + 1, 0:128], xin[:, cg * GW:(cg + 1) * GW])
                T.matmul(pl[:], s_wpk[0:FIN + 1, 128:256], xin[:, cg * GW:(cg + 1) * GW])
                A.activation(s_h0g_hi[:, sl], ph[:], AF.Relu)
                V.tensor_relu(s_h0g_lo[:, sl], pl[:])

    # ============ P2: edge pipeline + segment sum + root update =============
    # one block = EPC chunks of 128 edges, all scattering into 128 dst slots
    EB = EPC * 128                       # edges per block
    prw_ctx = tc.tile_pool(name="prw", bufs=1, space="PSUM")
    prw = prw_ctx.__enter__()
    rw1 = prw.tile([64, 257], F32, tag="rw1")   # Set2Set step 1: r = mean(out)
    with tc.tile_pool(name="p2zg", bufs=3) as pzg, \
         tc.tile_pool(name="p2ea", bufs=3) as pea, \
         tc.tile_pool(name="p2sb", bufs=5) as ps2, \
         tc.tile_pool(name="p2ags", bufs=2) as pag, \
         tc.tile_pool(name="p2eaw", bufs=2, space="PSUM") as peaw, \
         tc.tile_pool(name="p2tp", bufs=2, space="PSUM") as ptp, \
         tc.tile_pool(name="p2agg", bufs=1, space="PSUM") as pagg, \
         tc.tile_pool(name="p2out", bufs=1, space="PSUM") as pout:
        # stream tiles: fetch lazily, 2 blocks ahead (pool bufs=3 => at most
        # 3 generations alive, used strictly in emission order)
        xgts, eats = {}, {}

        def fetch(b):
            if b < NB and b not in xgts:
                xgt = pzg.tile([FIN + 1, EB], F16, tag="xgt")
                dma(xgt[:], xgT_d[:, b * EB:(b + 1) * EB])
                eat = pea.tile([FE + 1, EB], F16, tag="eat")
                dma(eat[:], eaT_d[:, b * EB:(b + 1) * EB])
                xgts[b], eats[b] = xgt, eat

        fetch(0)
        fetch(1)
        preload_rest()

        J2 = EPC // 2
        NQ = NB * J2
        tps = {}

        def emit_h0(q):
            """h0 for the 256 edges of sub-group q (software-pipelined)."""
            b, j2 = divmod(q, J2)
            psl = slice(j2 * 256, (j2 + 1) * 256)
            tp = ptp.tile([128, 2, 256], F32, tag="tp")
            T.matmul(tp[:, 0, :], s_wpk[0:FIN + 1, 0:128], xgts[b][:, psl])
            T.matmul(tp[:, 1, :], s_wpk[0:FIN + 1, 128:256], xgts[b][:, psl])
            tps[q] = tp

        emit_h0(0)
        pend = []                       # deferred scatter: (b, j, m1, S)
        aggs = {}

        def emit_tail(bb):
            # block tail: fold Wm2, root update, relu
            agh, agl = aggs.pop(bb)
            ags = pag.tile([128, 2, 128], F16, tag="ags")
            V.tensor_copy(ags[:, 0, :], agh[:])
            A.activation(ags[:, 1, :], agl[:], AF.Copy)
            po = pout.tile([128, D], F32, tag="po")
            csl = slice(bb * 128, (bb + 1) * 128)
            T.matmul(po[:], ags[:, 0, :], s_wm2hi[:], start=True, stop=False)
            T.matmul(po[:], ags[:, 1, :], s_wm2lo[:], start=False, stop=False)
            T.matmul(po[:], s_h0g_hi[:, csl], s_wrhi[:], start=False, stop=False)
            T.matmul(po[:], s_h0g_lo[:, csl], s_wrlo[:], start=False, stop=False)
            T.matmul(po[:], s_dego[:, csl], s_bmbr[:], start=False, stop=True)
            A.activation(s_out[:, bb, 0:D], po[:], AF.Relu)
            # Set2Set step 1 folds to a per-graph mean (bl==0 -> h=0, wt=1):
            # accumulate numerator/denominator inline while out[b] is hot
            T.matmul(rw1[:], s_G[:, bb, :], s_out[:, bb, :],
                     start=(bb == 0), stop=(bb == NB - 1))

        def flush_scatter():
            # scatter of the previous chunk (emitted mid-m1 of the current
            # one so PE never waits on the relu copy); accumulator tiles are
            # created/retired here so pagg bufs=1 stays alias-safe
            for (bb, jj, m1_, S_) in pend:
                if jj == 0:
                    agh = pagg.tile([128, 128], F32, tag="agh")
                    agl = pagg.tile([128, 128], F32, tag="agl")
                    aggs[bb] = (agh, agl)
                agh, agl = aggs[bb]
                T.matmul(agh[:], m1_[:, 0:128], S_[:],
                         start=(jj == 0), stop=(jj == EPC - 1))
                T.matmul(agl[:], m1_[:, 128:256], S_[:],
                         start=(jj == 0), stop=(jj == EPC - 1))
                if jj == EPC - 1:
                    emit_tail(bb)
            pend.clear()

        for q in range(NQ):
            b, j2 = divmod(q, J2)
            if j2 == 0:
                fetch(b + 2)
            if q + 1 < NQ:
                emit_h0(q + 1)          # PE runs ahead while copies drain
            tp = tps.pop(q)
            t16 = ps2.tile([128, 2, 256], F16, tag="t16")
            A.activation(t16[:, 0, :], tp[:, 0, :], AF.Relu)
            V.tensor_relu(t16[:, 1, :], tp[:, 1, :])
            for h in range(2):
                j = j2 * 2 + h
                i = b * EPC + j
                esl = slice(j * 128, (j + 1) * 128)
                hsl = slice(h * 128, (h + 1) * 128)
                S = ps2.tile([128, 128], F16, tag="S")
                GP.tensor_scalar(S[:], s_wpk[:, 1024:1152], s_dstcol[:, i:i + 1], None,
                                 op0=ALU.is_equal)
                pe_ = peaw.tile([128, D], F32, tag="eaw")
                T.matmul(pe_[:], eats[b][:, esl], s_wpk[0:FE + 1, 768:1024],
                         start=True, stop=False)
                T.matmul(pe_[:], t16[:, 0, hsl], s_wpk[:, 256:512],
                         start=False, stop=False)
                flush_scatter()         # scatter of j-1 lands mid-m1 of j
                T.matmul(pe_[:], t16[:, 1, hsl], s_wpk[:, 512:768],
                         start=False, stop=True)
                m1 = ps2.tile([128, D], F16, tag="m1")
                if j % 2 == 0:
                    A.activation(m1[:], pe_[:], AF.Relu)
                else:
                    V.tensor_relu(m1[:], pe_[:])
                pend.append((b, j, m1, S))
        flush_scatter()                 # last chunk's scatter + last tail

    # finish Set2Set step 1: r1 = rw1[:, :256] / rw1[:, 256]
    with tc.tile_pool(name="s1sb", bufs=1) as p1s, \
         tc.tile_pool(name="s1ps", bufs=2, space="PSUM") as p1p:
        rr1 = p1s.tile([64, 1], F32, tag="rr1")
        V.reciprocal(rr1[:], rw1[:, 256:257])
        rf1 = p1s.tile([64, D], F16, tag="rf1")
        V.tensor_scalar(rf1[:], rw1[:, 0:256], rr1[:], None, op0=ALU.mult)
        for mth in range(2):
            ptr = p1p.tile([128, 64], F16, tag="ptr")
            T.transpose(ptr[:], rf1[:, mth * 128:(mth + 1) * 128],
                        s_ident[0:64, 0:64])
            V.tensor_copy(s_rT[mth][:], ptr[:])
    prw_ctx.__exit__(None, None, None)

    # ============ P3: Set2Set (steps 2..3) + readout ========================
    with tc.tile_pool(name="p3ps", bufs=2, space="PSUM") as pp3, \
         tc.tile_pool(name="p3p1", bufs=1, space="PSUM") as pq3, \
         tc.tile_pool(name="p3g", bufs=1, space="PSUM") as pg3, \
         tc.tile_pool(name="p3sb", bufs=2) as ps3:
        for step in range(2):
            g0p = pg3.tile([64, 512], F32, tag="g0")
            g1p = pg3.tile([64, 512], F32, tag="g1")
            for half, gp in ((0, g0p), (1, g1p)):
                nsl = slice(half * 512, (half + 1) * 512)
                T.matmul(gp[:], s_ones1[:], s_blr[:, nsl], start=True, stop=False)
                for kk in range(4):
                    lhs = (s_hT + s_rT)[kk]
                    T.matmul(gp[:], lhs[:], s_wih[:, kk, nsl],
                             start=False, stop=False)
                for kk in range(2):
                    T.matmul(gp[:], s_hT[kk][:], s_whh[:, kk, nsl],
                             start=False, stop=(kk == 1))
            ti = ps3.tile([64, D], F32, tag="ti")
            tf = ps3.tile([64, D], F32, tag="tf")
            tg = ps3.tile([64, D], F32, tag="tg")
            to = ps3.tile([64, D], F32, tag="to")
            A.activation(ti[:], g0p[:, 0:256], AF.Tanh, scale=0.5)
            A.activation(tf[:], g0p[:, 256:512], AF.Tanh, scale=0.5)
            A.activation(tg[:], g1p[:, 0:256], AF.Tanh)
            A.activation(to[:], g1p[:, 256:512], AF.Tanh, scale=0.5)
            a2 = ps3.tile([64, D], F32, tag="a2")
            bv = ps3.tile([64, D], F32, tag="bv")
            V.scalar_tensor_tensor(a2[:], tf[:], 1.0, s_cu[:], ALU.add, ALU.mult)
            V.scalar_tensor_tensor(bv[:], ti[:], 1.0, tg[:], ALU.add, ALU.mult)
            V.scalar_tensor_tensor(s_cu[:], a2[:], 0.5, bv[:], ALU.mult, ALU.add)
            th = ps3.tile([64, D], F32, tag="th")
            A.activation(th[:], s_cu[:], AF.Tanh, scale=0.5)
            hh = ps3.tile([64, D], F32, tag="hh")
            V.scalar_tensor_tensor(hh[:], to[:], 1.0, th[:], ALU.add, ALU.mult)
            A.activation(s_hh16[:], hh[:], AF.Copy)
            for mth in range(2):
                ptr = pq3.tile([128, 64], F16, tag="ptr")
                T.transpose(ptr[:], s_hh16[:, mth * 128:(mth + 1) * 128],
                            s_ident[0:64, 0:64])
                V.tensor_copy(s_hT[mth][:], ptr[:])
            # attention: e, w=exp(e), r = (sum w*out)/(sum w)
            for c in range(NB):
                csl = slice(c * 128, (c + 1) * 128)
                hb = pp3.tile([128, D], F32, tag="hb")
                T.matmul(hb[:], s_GT[:, csl], s_hh16[:])
                scr = ps3.tile([128, D], F32, tag="scr")
                V.scalar_tensor_tensor(scr[:], s_out[:, c, 0:D], 0.5, hb[:],
                                       ALU.mult, ALU.mult,
                                       accum_out=s_e[:, c:c + 1])
            A.activation(s_wt[:], s_e[:], AF.Exp)
            rw = pg3.tile([64, 257], F32, tag="rw")
            for c in range(NB):
                gw = ps3.tile([128, 64], F16, tag="gw")
                GP.tensor_scalar(gw[:], s_G[:, c, :], s_wt[:, c:c + 1], None,
                                 op0=ALU.mult)
                T.matmul(rw[:], gw[:], s_out[:, c, :],
                         start=(c == 0), stop=(c == NB - 1))
            rr = ps3.tile([64, 1], F32, tag="rr")
            V.reciprocal(rr[:], rw[:, 256:257])
            rf = ps3.tile([64, D], F16, tag="rf")
            V.tensor_scalar(rf[:], rw[:, 0:256], rr[:], None, op0=ALU.mult)
            for mth in range(2):
                ptr = pq3.tile([128, 64], F16, tag="ptr")
                T.transpose(ptr[:], rf[:, mth * 128:(mth + 1) * 128],
                            s_ident[0:64, 0:64])
                V.tensor_copy(s_rT[mth][:], ptr[:])
        # readout
        for mth in range(2):
            yp = pq3.tile([128, 64], F32, tag="yp")
            for kk in range(4):
                T.matmul(yp[:], s_w1[:, kk, mth, :], (s_hT + s_rT)[kk][:],
                         start=(kk == 0), stop=(kk == 3))
            A.activation(s_y1[mth][:], yp[:], AF.Relu, bias=s_b1[:, mth:mth + 1])
        ypo = pq3.tile([64, 1], F32, tag="ypo")
        T.matmul(ypo[:], s_y1[0][:], s_w2[:, 0:1], start=True, stop=False)
        T.matmul(ypo[:], s_y1[1][:], s_w2[:, 1:2], start=False, stop=False)
        T.matmul(ypo[:], s_ones1[:], s_b2[:], start=False, stop=True)
        V.tensor_copy(s_yo[:], ypo[:])
        dma(y_d[:], s_yo[:])


_CACHE = {}


def _get_compiled(NB, EPC):
    key = (NB, EPC)
    if key not in _CACHE:
        nc = bacc.Bacc("TRN2", target_bir_lowering=False, debug=False,
                       num_devices=N_CORES)
        with tile.TileContext(nc) as tc:
            _build(nc, tc, NB, EPC)
        nc.compile()
        _CACHE[key] = nc
    return _CACHE[key]


def kernel(**inputs) -> np.ndarray:
    in_maps, NB, EPC = _host_prep(inputs)
    nc = _get_compiled(NB, EPC)
    res = run_bass_kernel_spmd(nc, in_maps, list(range(N_CORES)))
    y = np.concatenate([res.results[k]["y"].reshape(-1) for k in range(N_CORES)])
    return y.astype(np.float32)
